# revision 1
# baseline (speedup 1.0000x reference)
"""BitLinear (ternary-weight / int8-activation quantized linear) on 8 trn2 NeuronCores.

Math (matches the jax reference up to fp32 rounding):
    eta   = clip(max|x| along k, 1e-5)             per row
    x_q   = round(x * 127 / eta)    in [-127,127]  (round-half-even)
    gamma = clip(mean|w|, 1e-5)                    scalar
    w_q   = round(clip(w / gamma, -1, 1))          in {-1,0,1}
    out   = (x_q @ w_q^T) * (eta/127 * gamma) + bias

x_q / w_q are small integers exactly representable in bf16 and the PE
accumulates in fp32, so the bf16 matmul is EXACT.  Rounding uses the fp32
magic-number trick  rint(t) = (t + 1.5*2^23) - 1.5*2^23  (round-half-even).

Sharding: data-parallel over rows of x (16384 -> 2048 rows/core), weight+bias
replicated.  Per-core schedule:
  phase W: stream w once (both HWDGE queues), fused |w| reduce -> gamma,
           quantize, PE-transpose into k-major wqT (SBUF resident, bf16)
  phase X: stream x, per-row eta, quantize, round-trip x_q through DRAM;
           m-block 0 transposed on PE, blocks 1-3 via DMA-xbar transposed
           loads that overlap the matmul phase
  phase MM: 1024 bf16 matmuls (k-contiguous per m-tile), ACT dequant-scale
           from PSUM, DVE bias add, stores on the sync queue
"""

import os
from contextlib import ExitStack

import numpy as np
import ml_dtypes

import concourse.bass as bass
import concourse.bacc as bacc
import concourse.mybir as mybir
import concourse.tile as tile
from concourse.bass_utils import run_bass_kernel_spmd

P = 128
K = 2048
N = 2048
M_CORE = 2048
KT = K // P          # 16
NT = N // P          # 16
MT = M_CORE // P     # 16
NBLK = N // 512      # 4
N_CORES = 8
C_MAGIC = 12582912.0     # 1.5 * 2**23
INV_NK = 1.0 / (N * K)

F32 = mybir.dt.float32
BF16 = mybir.dt.bfloat16
ALU = mybir.AluOpType
AXIS = mybir.AxisListType
ACTF = mybir.ActivationFunctionType


def _build_program():
    nc = bacc.Bacc("TRN2", target_bir_lowering=False, debug=False)

    x_d = nc.dram_tensor("x", [M_CORE, K], F32, kind="ExternalInput").ap()
    w_d = nc.dram_tensor("weight", [N, K], F32, kind="ExternalInput").ap()
    b_d = nc.dram_tensor("bias", [1, N], F32, kind="ExternalInput").ap()
    out_d = nc.dram_tensor("out", [M_CORE, N], F32, kind="ExternalOutput").ap()
    xq_rt_d = nc.dram_tensor("xq_rt", [M_CORE, K], BF16).ap()
    ident_d = nc.inline_tensor(
        np.eye(P, dtype=ml_dtypes.bfloat16), name="ident128"
    ).ap()
    identf_d = nc.inline_tensor(
        np.eye(P, dtype=np.float32), name="ident128f"
    ).ap()

    with tile.TileContext(nc) as tc, ExitStack() as ctx:
        consts = ctx.enter_context(tc.tile_pool(name="consts", bufs=1))
        stats = ctx.enter_context(tc.tile_pool(name="stats", bufs=1))
        wqT_p = ctx.enter_context(tc.tile_pool(name="wqT", bufs=1))
        ps_tr = ctx.enter_context(
            tc.tile_pool(name="pstr", bufs=2, space=bass.MemorySpace.PSUM)
        )
        ps_mm = ctx.enter_context(
            tc.tile_pool(name="psmm", bufs=5, space=bass.MemorySpace.PSUM)
        )

        # ---- constants / stats ----
        ident_sb = consts.tile([P, P], BF16)
        nc.sync.dma_start(ident_sb[:], ident_d[:, :])
        ones128 = consts.tile([P, P], F32)
        nc.vector.memset(ones128[:], 1.0)
        ident_f32 = consts.tile([P, P], F32)
        nc.sync.dma_start(ident_f32[:], identf_d[:, :])

        eta_raw = stats.tile([P, MT], F32)
        eta_all = stats.tile([P, MT], F32)
        inv_eta = stats.tile([P, MT], F32)
        qs_all = stats.tile([P, MT], F32)
        osc_all = stats.tile([P, MT], F32)
        wparts = stats.tile([P, NT], F32)
        wsum = stats.tile([P, 1], F32)
        gamma = stats.tile([P, 1], F32)
        inv_g = stats.tile([P, 1], F32)

        # k-major quantized operands, single big tiles:
        # layout [128 k-part, kt*2048 + col]
        wqT_all = wqT_p.tile([P, KT * N], BF16)
        wqT_3d = wqT_all[:].rearrange("p (t n) -> p t n", t=KT)

        # ============ gamma: streamed |w| reduce (pass 1, tiles discarded) ====
        with tc.tile_pool(name="w1stage", bufs=4) as w1stage:
            for nt in range(NT):
                t = w1stage.tile([P, K], F32, tag="w1", name=f"w1_{nt}")
                eng = nc.sync if nt % 2 == 0 else nc.scalar
                eng.dma_start(t[:], w_d[nt * P:(nt + 1) * P, :])
                nc.vector.tensor_reduce(
                    wparts[:, nt:nt + 1], t[:], axis=AXIS.X, op=ALU.add,
                    apply_absolute_value=True,
                )
        nc.vector.tensor_reduce(wsum[:], wparts[:], axis=AXIS.X, op=ALU.add)
        pg = ps_mm.tile([P, 1], F32, tag="psg", name="psg", bufs=1)
        nc.tensor.matmul(pg[:], ones128[:, :], wsum[:])
        nc.vector.tensor_scalar(
            gamma[:], pg[:], scalar1=INV_NK, scalar2=1e-5,
            op0=ALU.mult, op1=ALU.max,
        )
        nc.vector.reciprocal(inv_g[:], gamma[:])

        # =================== phase W (streamed quantize) ===================
        with tc.tile_pool(name="wstage", bufs=5) as wstage, \
             tc.tile_pool(name="wqst", bufs=3) as wqst:
            for nt in range(NT):
                t = wstage.tile([P, K], F32, tag="w", name=f"w{nt}")
                eng = nc.sync if nt % 2 == 0 else nc.scalar
                eng.dma_start(t[:], w_d[nt * P:(nt + 1) * P, :])
                # t = w/gamma + C on ACT (fp32 store rounds to the integer
                # grid; round-then-clip == clip-then-round for this quantizer)
                nc.scalar.activation(
                    t[:], t[:], ACTF.Copy, bias=C_MAGIC, scale=inv_g[:, :]
                )
                nc.vector.tensor_scalar(
                    t[:], t[:], scalar1=C_MAGIC, scalar2=1.0,
                    op0=ALU.subtract, op1=ALU.min,
                )
                q = wqst.tile([P, K], BF16, tag="wq", name=f"wq{nt}")
                nc.vector.tensor_scalar(
                    q[:], t[:], scalar1=-1.0, scalar2=None, op0=ALU.max,
                )
                for g in range(4):
                    pt = ps_tr.tile([P, 512], BF16, tag="ptr", name=f"wt{nt}_{g}")
                    for j in range(4):
                        kt = g * 4 + j
                        nc.tensor.transpose(
                            pt[:, j * P:(j + 1) * P],
                            q[:, kt * P:(kt + 1) * P],
                            ident_sb[:],
                        )
                    dst = wqT_3d[:, g * 4:(g + 1) * 4, nt * P:(nt + 1) * P]
                    src = pt[:].rearrange("p (j n) -> p j n", j=4)
                    if g % 2 == 0:
                        nc.scalar.copy(dst, src)
                    else:
                        nc.vector.tensor_copy(dst, src)

        # =================== phase X + MM (pipelined) ===================
        with tc.tile_pool(name="xqT", bufs=1) as xqT_p, \
             tc.tile_pool(name="xstage", bufs=4) as xstage, \
             tc.tile_pool(name="xqst", bufs=4) as xqst, \
             tc.tile_pool(name="bias_p", bufs=1) as bias_p, \
             tc.tile_pool(name="outst", bufs=3) as outst:
            xqT_all = xqT_p.tile([P, KT * M_CORE], BF16)
            xqT_3d = xqT_all[:].rearrange("p (t m) -> p t m", t=KT)
            b_row = bias_p.tile([1, N], F32)
            nc.sync.dma_start(b_row[:], b_d[:, :])
            b_bf = bias_p.tile([1, N], BF16)
            nc.vector.tensor_copy(b_bf[:], b_row[:])
            # per-mt inv_osc rows (bf16, on partition 0) for the bias matmul
            ios_row = bias_p.tile([1, MT * P], BF16)
            ios_f32 = bias_p.tile([1, MT * P], F32)

            xq_tiles = {}

            def x_iter(mt, store_rt):
                t = xstage.tile([P, K], F32, tag="x", name=f"x{mt}")
                nc.gpsimd.dma_start(t[:], x_d[mt * P:(mt + 1) * P, :])
                nc.vector.tensor_reduce(
                    eta_raw[:, mt:mt + 1], t[:], axis=AXIS.X, op=ALU.max,
                    apply_absolute_value=True,
                )
                nc.vector.tensor_scalar(
                    eta_all[:, mt:mt + 1], eta_raw[:, mt:mt + 1],
                    scalar1=1e-5, scalar2=None, op0=ALU.max,
                )
                nc.vector.reciprocal(inv_eta[:, mt:mt + 1], eta_all[:, mt:mt + 1])
                nc.vector.tensor_scalar(
                    qs_all[:, mt:mt + 1], inv_eta[:, mt:mt + 1],
                    scalar1=127.0, scalar2=None, op0=ALU.mult,
                )
                nc.scalar.activation(
                    t[:], t[:], ACTF.Copy, bias=C_MAGIC,
                    scale=qs_all[:, mt:mt + 1],
                )
                q = xqst.tile([P, K], BF16, tag="xq", name=f"xq{mt}")
                nc.vector.tensor_scalar(
                    q[:], t[:], scalar1=C_MAGIC, scalar2=None, op0=ALU.subtract,
                )
                xq_tiles[mt] = q
                if store_rt:
                    nc.scalar.dma_start(xq_rt_d[mt * P:(mt + 1) * P, :], q[:])
                # inv_osc row: transpose inv_eta column, scale by 127/gamma
                pt = ps_tr.tile([1, P], F32, tag="ptr", name=f"ios{mt}")
                nc.tensor.transpose(pt[:], inv_eta[:, mt:mt + 1], ident_f32[:])
                rs = slice(mt * P, (mt + 1) * P)
                nc.scalar.copy(ios_f32[:, rs], pt[:])
                nc.vector.tensor_scalar(
                    ios_f32[:, rs], ios_f32[:, rs],
                    scalar1=inv_g[0:1, :], scalar2=127.0,
                    op0=ALU.mult, op1=ALU.mult,
                )
                nc.vector.tensor_copy(ios_row[:, rs], ios_f32[:, rs])

            def x_transpose(mt):
                q = xq_tiles[mt]
                for g in range(4):
                    pt = ps_tr.tile([P, 512], BF16, tag="ptr", name=f"xt{mt}_{g}")
                    for j in range(4):
                        kt = g * 4 + j
                        nc.tensor.transpose(
                            pt[:, j * P:(j + 1) * P],
                            q[:, kt * P:(kt + 1) * P],
                            ident_sb[:],
                        )
                    dst = xqT_3d[:, g * 4:(g + 1) * 4, mt * P:(mt + 1) * P]
                    src = pt[:].rearrange("p (j m) -> p j m", j=4)
                    if g % 2 == 0:
                        nc.scalar.copy(dst, src)
                    else:
                        nc.vector.tensor_copy(dst, src)

            def xbar_load(r0, r1):
                for kt in range(KT):
                    nc.sync.dma_start_transpose(
                        xqT_3d[:, kt, r0:r1],
                        xq_rt_d[r0:r1, kt * P:(kt + 1) * P],
                    )

            def mm_block(mt):
                nc.vector.tensor_scalar(
                    osc_all[:, mt:mt + 1], eta_all[:, mt:mt + 1],
                    scalar1=gamma[:, :], scalar2=1.0 / 127.0,
                    op0=ALU.mult, op1=ALU.mult,
                )
                pss = [
                    ps_mm.tile([P, 512], F32, tag="psmm", name=f"ps{mt}_{nb}")
                    for nb in range(NBLK)
                ]
                for kt in range(KT):
                    lhsT = xqT_3d[:, kt, mt * P:(mt + 1) * P]
                    for nb in range(NBLK):
                        nc.tensor.matmul(
                            pss[nb][:],
                            lhsT,
                            wqT_3d[:, kt, nb * 512:(nb + 1) * 512],
                            start=(kt == 0),
                            stop=False,
                        )
                # bias as a rank-1 K=1 accumulation: psum += inv_osc[m]*bias[n]
                for nb in range(NBLK):
                    nc.tensor.matmul(
                        pss[nb][:],
                        ios_row[:, mt * P:(mt + 1) * P],
                        b_bf[:, nb * 512:(nb + 1) * 512],
                        start=False,
                        stop=True,
                    )
                for nb in range(NBLK):
                    o = outst.tile([P, 512], F32, tag="o", name=f"o{mt}_{nb}")
                    nc.scalar.activation(
                        o[:], pss[nb][:], ACTF.Copy, bias=0.0,
                        scale=osc_all[:, mt:mt + 1],
                    )
                    nc.sync.dma_start(
                        out_d[mt * P:(mt + 1) * P, nb * 512:(nb + 1) * 512], o[:]
                    )

            # software-pipelined: x chain runs one m-block ahead of matmuls
            for mt in range(MT):
                x_iter(mt, store_rt=(mt >= 4))
                if mt < 4:
                    x_transpose(mt)
                if mt == 7:
                    xbar_load(512, 1024)
                elif mt == 11:
                    xbar_load(1024, 1536)
                elif mt == 15:
                    xbar_load(1536, 2048)
                if mt >= 4:
                    mm_block(mt - 4)
            for mt in range(MT - 4, MT):
                mm_block(mt)
    nc.compile()
    return nc


_NC_CACHE = None
LAST_EXEC_NS = None


def _get_nc():
    global _NC_CACHE
    if _NC_CACHE is None:
        _NC_CACHE = _build_program()
    return _NC_CACHE


def _make_in_maps(x, weight, bias):
    xf = np.ascontiguousarray(np.asarray(x, dtype=np.float32).reshape(-1, K))
    w = np.ascontiguousarray(np.asarray(weight, dtype=np.float32))
    b = np.ascontiguousarray(np.asarray(bias, dtype=np.float32).reshape(1, N))
    assert xf.shape[0] == N_CORES * M_CORE
    return [
        {
            "x": xf[c * M_CORE:(c + 1) * M_CORE],
            "weight": w,
            "bias": b,
        }
        for c in range(N_CORES)
    ]


def kernel(x, weight, bias):
    global LAST_EXEC_NS
    nc = _get_nc()
    in_maps = _make_in_maps(x, weight, bias)
    trace = bool(int(os.environ.get("BITLINEAR_TRACE", "0")))
    res = run_bass_kernel_spmd(nc, in_maps, list(range(N_CORES)), trace=trace)
    LAST_EXEC_NS = res.exec_time_ns
    out = np.concatenate([res.results[c]["out"] for c in range(N_CORES)], axis=0)
    return out.reshape(np.asarray(x).shape[:-1] + (N,)).astype(np.float32)



# revision 8
# speedup vs baseline: 1.2589x; 1.2589x over previous
"""BitLinear (ternary-weight / int8-activation quantized linear) on 8 trn2 NeuronCores.

Math (matches the jax reference up to fp32 rounding):
    eta   = clip(max|x| along k, 1e-5)             per row
    x_q   = round(x * 127 / eta)    in [-127,127]  (round-half-even)
    gamma = clip(mean|w|, 1e-5)                    scalar
    w_q   = round(clip(w / gamma, -1, 1))          in {-1,0,1}
    out   = (x_q @ w_q^T) * (eta/127 * gamma) + bias

x_q / w_q are small integers exactly representable in bf16 and the PE
accumulates in fp32, so the bf16 matmul is EXACT.  Rounding uses the fp32
magic-number trick  rint(t) = (t + 1.5*2^23) - 1.5*2^23  (round-half-even).
The w clip is done BEFORE scaling:  round(clip(w/g,-1,1)) == round(clip(w,-g,g)/g)
for any g>0 (elements |w|>=g map to +-1 either way), which saves a full pass.

Sharding: data-parallel over rows of x (16384 -> 2048 rows/core), weight+bias
replicated.  gamma = mean|w| is computed with an 8-way AllReduce collective:
each core |.|-reduces only its 256-row shard of w (host-sliced input w_shard),
so the full-w stream only has to happen ONCE per core and the scalar gamma is
ready ~25us in instead of ~50us.

Per-core schedule (all phases overlap):
  S:  DMA w_shard (2 tiles), |.|-reduce, AllReduce -> gamma, inv_g, -gamma
  W:  stream all 16 w tiles once (HWDGE, starts at t=0, buffered bufs=7);
      per tile: clip to [-g,g] (DVE), *inv_g + C magic-round (ACT),
      PE fp32-transpose, PSUM->SBUF copy with fused -C -> wqT bf16
  X:  per tile: DMA (SWDGE), abs-max -> eta (GP early / DVE late), *qs + C
      (ACT), -C -> bf16 (ACT/DVE), PE bf16-transpose -> rolling xqT pool
  MM: single mt-outer sweep, 16 kt x 4 nb bf16 matmuls -> 4 PSUM banks,
      fused dequant+bias (scalar_tensor_tensor: psum*osc + bias_bcast) -> out
"""

import os
from contextlib import ExitStack

import numpy as np
import ml_dtypes

import concourse.bass as bass
import concourse.bacc as bacc
import concourse.mybir as mybir
import concourse.tile as tile
from concourse.bass_utils import run_bass_kernel_spmd

P = 128
K = 2048
N = 2048
M_CORE = 2048
KT = K // P          # 16
NT = N // P          # 16
MT = M_CORE // P     # 16
NBLK = N // 512      # 4
N_CORES = 8
SHARD_NT = 2         # w_shard rows / 128
C_MAGIC = 12582912.0     # 1.5 * 2**23
INV_NK = 1.0 / (N * K)
USE_COLLECTIVE = True
N_XPRO = 3           # x tiles processed on GP/ACT before the mm loop

F32 = mybir.dt.float32
BF16 = mybir.dt.bfloat16
ALU = mybir.AluOpType
AXIS = mybir.AxisListType
ACTF = mybir.ActivationFunctionType


def _build_program():
    nc = bacc.Bacc(
        "TRN2", target_bir_lowering=False, debug=False,
        num_devices=N_CORES if USE_COLLECTIVE else None,
    )

    x_d = nc.dram_tensor("x", [M_CORE, K], F32, kind="ExternalInput").ap()
    w_d = nc.dram_tensor("weight", [N, K], F32, kind="ExternalInput").ap()
    ws_d = nc.dram_tensor("w_shard", [SHARD_NT * P, K], F32, kind="ExternalInput").ap()
    b_d = nc.dram_tensor("bias", [1, N], F32, kind="ExternalInput").ap()
    out_d = nc.dram_tensor("out", [M_CORE, N], F32, kind="ExternalOutput").ap()
    ident_d = nc.inline_tensor(
        np.eye(P, dtype=ml_dtypes.bfloat16), name="ident128"
    ).ap()
    identf_d = nc.inline_tensor(
        np.eye(P, dtype=np.float32), name="ident128f"
    ).ap()

    with tile.TileContext(nc) as tc, ExitStack() as ctx:
        consts = ctx.enter_context(tc.tile_pool(name="consts", bufs=1))
        stats = ctx.enter_context(tc.tile_pool(name="stats", bufs=1))
        bias_p = ctx.enter_context(tc.tile_pool(name="bias_p", bufs=1))
        wqT_p = ctx.enter_context(tc.tile_pool(name="wqT", bufs=1))
        xqT_p = ctx.enter_context(tc.tile_pool(name="xqT", bufs=6))
        wstage = ctx.enter_context(tc.tile_pool(name="wstage", bufs=7))
        xstage = ctx.enter_context(tc.tile_pool(name="xstage", bufs=3))
        xqst = ctx.enter_context(tc.tile_pool(name="xqst", bufs=2))
        outst = ctx.enter_context(tc.tile_pool(name="outst", bufs=3))
        ps_tr = ctx.enter_context(
            tc.tile_pool(name="pstr", bufs=2, space=bass.MemorySpace.PSUM)
        )
        ps_mm = ctx.enter_context(
            tc.tile_pool(name="psmm", bufs=6, space=bass.MemorySpace.PSUM)
        )

        # ---- constants ----
        ident_sb = consts.tile([P, P], BF16)
        nc.sync.dma_start(ident_sb[:], ident_d[:, :])
        identf_sb = consts.tile([P, P], F32)
        nc.sync.dma_start(identf_sb[:], identf_d[:, :])
        ones128 = consts.tile([P, P], F32)
        nc.vector.memset(ones128[:], 1.0)
        onesrow = consts.tile([1, P], F32)
        nc.vector.memset(onesrow[:], 1.0)
        b_row = consts.tile([1, N], F32)
        nc.sync.dma_start(b_row[:], b_d[:, :])

        wparts = stats.tile([P, NT], F32)
        wsum = stats.tile([P, 1], F32)
        gamma = stats.tile([P, 1], F32)
        inv_g = stats.tile([P, 1], F32)
        neg_g = stats.tile([P, 1], F32)
        eta_raw = stats.tile([P, MT], F32)
        eta_all = stats.tile([P, MT], F32)
        inv_eta = stats.tile([P, MT], F32)
        qs_all = stats.tile([P, MT], F32)
        osc_all = stats.tile([P, MT], F32)

        bias_bcast = bias_p.tile([P, N], F32)

        # k-major quantized weights: [128 k-part, kt*2048 + n]
        wqT_all = wqT_p.tile([P, KT * N], BF16)
        wqT_3d = wqT_all[:].rearrange("p (t n) -> p t n", t=KT)

        # ============ phase S: gamma ============
        x_stage_tiles = {}

        def x_dma(mt):
            t = xstage.tile([P, K], F32, tag="x", name=f"x{mt}")
            nc.gpsimd.dma_start(t[:], x_d[mt * P:(mt + 1) * P, :])
            x_stage_tiles[mt] = t

        if USE_COLLECTIVE:
            dram = ctx.enter_context(
                tc.tile_pool(name="ccdram", bufs=1, space="DRAM")
            )
            cc_in = dram.tile([P, 1], F32)
            cc_out = dram.tile([P, 1], F32)
            for st in range(SHARD_NT):
                t = wstage.tile([P, K], F32, tag="w", name=f"wsh{st}")
                nc.gpsimd.dma_start(t[:], ws_d[st * P:(st + 1) * P, :])
                nc.vector.tensor_reduce(
                    wparts[:, st:st + 1], t[:], axis=AXIS.X, op=ALU.add,
                    apply_absolute_value=True,
                )
            # x prologue DMA triggers go on the GP queue ahead of the
            # collective chain so they fire at t~0.
            for mt in range(N_XPRO):
                x_dma(mt)
            nc.vector.tensor_reduce(
                wsum[:], wparts[:, 0:SHARD_NT], axis=AXIS.X, op=ALU.add
            )
            nc.gpsimd.dma_start(cc_in[:], wsum[:])
            nc.gpsimd.collective_compute(
                "AllReduce",
                ALU.add,
                replica_groups=[list(range(N_CORES))],
                ins=[cc_in.opt()],
                outs=[cc_out.opt()],
            )
            # readback trigger waits for the collective (~20us). It must not
            # sit on the sync/scalar HWDGE rings (would stall the w stream);
            # the GP queue only has the in-loop x DMA triggers behind it,
            # which have tens of us of slack.
            self_cc_out = cc_out
        else:
            for nt in range(NT):
                t = wstage.tile([P, K], F32, tag="w", name=f"wp1_{nt}")
                eng = nc.sync if nt % 2 == 0 else nc.scalar
                eng.dma_start(t[:], w_d[nt * P:(nt + 1) * P, :])
                nc.vector.tensor_reduce(
                    wparts[:, nt:nt + 1], t[:], axis=AXIS.X, op=ALU.add,
                    apply_absolute_value=True,
                )
            nc.vector.tensor_reduce(wsum[:], wparts[:], axis=AXIS.X, op=ALU.add)

        # ---- x prologue tiles (GP reduce + ACT quantize; not gamma-gated) ----
        xq_tiles = {}
        xqT_tiles = {}

        def x_head(mt, early):
            if mt in x_stage_tiles:
                t = x_stage_tiles.pop(mt)
            else:
                t = xstage.tile([P, K], F32, tag="x", name=f"x{mt}")
                nc.gpsimd.dma_start(t[:], x_d[mt * P:(mt + 1) * P, :])
            nc.vector.tensor_reduce(
                eta_raw[:, mt:mt + 1], t[:], axis=AXIS.X, op=ALU.max,
                apply_absolute_value=True,
            )
            nc.vector.tensor_scalar(
                eta_all[:, mt:mt + 1], eta_raw[:, mt:mt + 1],
                scalar1=1e-5, scalar2=None, op0=ALU.max,
            )
            nc.vector.reciprocal(inv_eta[:, mt:mt + 1], eta_all[:, mt:mt + 1])
            nc.vector.tensor_scalar(
                qs_all[:, mt:mt + 1], inv_eta[:, mt:mt + 1],
                scalar1=127.0, scalar2=None, op0=ALU.mult,
            )
            nc.scalar.activation(
                t[:], t[:], ACTF.Copy, bias=C_MAGIC,
                scale=qs_all[:, mt:mt + 1],
            )
            q = xqst.tile([P, K], BF16, tag="xq", name=f"xq{mt}")
            if early:
                nc.scalar.activation(q[:], t[:], ACTF.Copy, bias=-C_MAGIC)
            else:
                nc.vector.tensor_scalar(
                    q[:], t[:], scalar1=C_MAGIC, scalar2=None, op0=ALU.subtract,
                )
            xq_tiles[mt] = q

        def x_transpose(mt, early):
            q = xq_tiles.pop(mt)
            xqTm = xqT_p.tile([P, KT * P], BF16, tag="xqT", name=f"xqT{mt}")
            xqT3 = xqTm[:].rearrange("p (t m) -> p t m", t=KT)
            for g in range(4):
                pt = ps_tr.tile([P, 512], BF16, tag="ptr", name=f"xt{mt}_{g}")
                for j in range(4):
                    kt = g * 4 + j
                    nc.tensor.transpose(
                        pt[:, j * P:(j + 1) * P],
                        q[:, kt * P:(kt + 1) * P],
                        ident_sb[:],
                    )
                dst = xqT3[:, g * 4:(g + 1) * 4, :]
                src = pt[:].rearrange("p (j m) -> p j m", j=4)
                if early or g % 2 == 0:
                    nc.scalar.copy(dst, src)
                else:
                    nc.vector.tensor_copy(dst, src)
            xqT_tiles[mt] = xqT3

        for mt in range(N_XPRO):
            x_head(mt, early=True)
            x_transpose(mt, early=True)

        if USE_COLLECTIVE:
            nc.gpsimd.dma_start(wsum[:], self_cc_out[:])

        # ---- gamma epilogue (PE fold + scalars) ----
        pg = ps_mm.tile([P, 1], F32, tag="psmm", name="psg")
        nc.tensor.matmul(pg[:], ones128[:, :], wsum[:])
        nc.vector.tensor_scalar(
            gamma[:], pg[:], scalar1=INV_NK, scalar2=1e-5,
            op0=ALU.mult, op1=ALU.max,
        )
        nc.vector.reciprocal(inv_g[:], gamma[:])
        nc.vector.tensor_scalar(
            neg_g[:], gamma[:], scalar1=-1.0, scalar2=None, op0=ALU.mult,
        )

        # ---- bias broadcast to all partitions: psum = ones^T (x) bias ----
        for nb in range(NBLK):
            ps = ps_mm.tile([P, 512], F32, tag="psmm", name=f"psb{nb}")
            nc.tensor.matmul(
                ps[:], onesrow[:, :], b_row[:, nb * 512:(nb + 1) * 512]
            )
            nc.scalar.copy(bias_bcast[:, nb * 512:(nb + 1) * 512], ps[:])

        # ============ phase W: stream + quantize + transpose w ============
        for nt in range(NT):
            t = wstage.tile([P, K], F32, tag="w", name=f"w{nt}")
            eng = nc.sync if nt % 2 == 0 else nc.scalar
            eng.dma_start(t[:], w_d[nt * P:(nt + 1) * P, :])
            # clip(w, -g, g): round(clip(w/g,-1,1)) == round(clip(w,-g,g)/g)
            nc.vector.tensor_scalar(
                t[:], t[:], scalar1=gamma[:, :], scalar2=neg_g[:, :],
                op0=ALU.min, op1=ALU.max,
            )
            # t = w_clip * inv_g + C: fp32 store rounds to the integer grid
            nc.scalar.activation(
                t[:], t[:], ACTF.Copy, bias=C_MAGIC, scale=inv_g[:, :]
            )
            for g in range(4):
                pt = ps_tr.tile([P, 512], F32, tag="ptr", name=f"wt{nt}_{g}")
                for j in range(4):
                    kt = g * 4 + j
                    nc.tensor.transpose(
                        pt[:, j * P:(j + 1) * P],
                        t[:, kt * P:(kt + 1) * P],
                        identf_sb[:],
                    )
                dst = wqT_3d[:, g * 4:(g + 1) * 4, nt * P:(nt + 1) * P]
                src = pt[:].rearrange("p (j n) -> p j n", j=4)
                if g % 3 == 0:
                    nc.scalar.activation(dst, src, ACTF.Copy, bias=-C_MAGIC)
                else:
                    nc.vector.tensor_scalar(
                        dst, src, scalar1=C_MAGIC, scalar2=None,
                        op0=ALU.subtract,
                    )

        # ============ phase MM: single mt-outer sweep ============
        def mm_block(mt):
            nc.vector.tensor_scalar(
                osc_all[:, mt:mt + 1], eta_all[:, mt:mt + 1],
                scalar1=gamma[:, :], scalar2=1.0 / 127.0,
                op0=ALU.mult, op1=ALU.mult,
            )
            xqT3 = xqT_tiles.pop(mt)
            pss = [
                ps_mm.tile([P, 512], F32, tag="psmm", name=f"ps{mt}_{nb}")
                for nb in range(NBLK)
            ]
            for kt in range(KT):
                lhsT = xqT3[:, kt, :]
                for nb in range(NBLK):
                    nc.tensor.matmul(
                        pss[nb][:],
                        lhsT,
                        wqT_3d[:, kt, nb * 512:(nb + 1) * 512],
                        start=(kt == 0),
                        stop=(kt == KT - 1),
                    )
            for nb in range(NBLK):
                o = outst.tile([P, 512], F32, tag="o", name=f"o{mt}_{nb}")
                nc.vector.scalar_tensor_tensor(
                    o[:], pss[nb][:], osc_all[:, mt:mt + 1],
                    bias_bcast[:, nb * 512:(nb + 1) * 512],
                    op0=ALU.mult, op1=ALU.add,
                )
                eng = nc.sync if nb % 2 == 0 else nc.scalar
                eng.dma_start(
                    out_d[mt * P:(mt + 1) * P, nb * 512:(nb + 1) * 512], o[:]
                )

        for mt in range(MT):
            nxt = mt + N_XPRO
            if nxt < MT:
                x_head(nxt, early=False)
                x_transpose(nxt, early=False)
            mm_block(mt)
    nc.compile()
    return nc


_NC_CACHE = None
LAST_EXEC_NS = None


def _get_nc():
    global _NC_CACHE
    if _NC_CACHE is None:
        _NC_CACHE = _build_program()
    return _NC_CACHE


def _make_in_maps(x, weight, bias):
    xf = np.ascontiguousarray(np.asarray(x, dtype=np.float32).reshape(-1, K))
    w = np.ascontiguousarray(np.asarray(weight, dtype=np.float32))
    b = np.ascontiguousarray(np.asarray(bias, dtype=np.float32).reshape(1, N))
    assert xf.shape[0] == N_CORES * M_CORE
    rows = SHARD_NT * P
    return [
        {
            "x": xf[c * M_CORE:(c + 1) * M_CORE],
            "weight": w,
            "w_shard": np.ascontiguousarray(w[c * rows:(c + 1) * rows]),
            "bias": b,
        }
        for c in range(N_CORES)
    ]


def kernel(x, weight, bias):
    global LAST_EXEC_NS
    nc = _get_nc()
    in_maps = _make_in_maps(x, weight, bias)
    trace = bool(int(os.environ.get("BITLINEAR_TRACE", "0")))
    res = run_bass_kernel_spmd(nc, in_maps, list(range(N_CORES)), trace=trace)
    LAST_EXEC_NS = res.exec_time_ns
    out = np.concatenate([res.results[c]["out"] for c in range(N_CORES)], axis=0)
    return out.reshape(np.asarray(x).shape[:-1] + (N,)).astype(np.float32)


# revision 11
# speedup vs baseline: 1.2726x; 1.0109x over previous
"""BitLinear (ternary-weight / int8-activation quantized linear) on 8 trn2 NeuronCores.

Math (matches the jax reference up to fp32 rounding):
    eta   = clip(max|x| along k, 1e-5)             per row
    x_q   = round(x * 127 / eta)    in [-127,127]  (round-half-even)
    gamma = clip(mean|w|, 1e-5)                    scalar
    w_q   = round(clip(w / gamma, -1, 1))          in {-1,0,1}
    out   = (x_q @ w_q^T) * (eta/127 * gamma) + bias

x_q / w_q are small integers exactly representable in bf16 and the PE
accumulates in fp32, so the bf16 matmul is EXACT.  Rounding uses the fp32
magic-number trick  rint(t) = (t + 1.5*2^23) - 1.5*2^23  (round-half-even).
The w clip is done BEFORE scaling:  round(clip(w/g,-1,1)) == round(clip(w,-g,g)/g)
for any g>0 (elements |w|>=g map to +-1 either way), which saves a full pass.

Sharding: data-parallel over rows of x (16384 -> 2048 rows/core), weight+bias
replicated.  gamma = mean|w| is computed with an 8-way AllReduce collective:
each core |.|-reduces only its 256-row shard of w (host-sliced input w_shard),
so the full-w stream only has to happen ONCE per core and the scalar gamma is
ready ~25us in instead of ~50us.

Per-core schedule (all phases overlap):
  S:  DMA w_shard (2 tiles), |.|-reduce, AllReduce -> gamma, inv_g, -gamma
  W:  stream all 16 w tiles once (HWDGE, starts at t=0, buffered bufs=7);
      per tile: clip to [-g,g] (DVE), *inv_g + C magic-round (ACT),
      PE fp32-transpose, PSUM->SBUF copy with fused -C -> wqT bf16
  X:  per tile: DMA (SWDGE), abs-max -> eta (GP early / DVE late), *qs + C
      (ACT), -C -> bf16 (ACT/DVE), PE bf16-transpose -> rolling xqT pool
  MM: single mt-outer sweep, 16 kt x 4 nb bf16 matmuls -> 4 PSUM banks,
      fused dequant+bias (scalar_tensor_tensor: psum*osc + bias_bcast) -> out
"""

import os
from contextlib import ExitStack

import numpy as np
import ml_dtypes

import concourse.bass as bass
import concourse.bacc as bacc
import concourse.mybir as mybir
import concourse.tile as tile
from concourse.bass_utils import run_bass_kernel_spmd

P = 128
K = 2048
N = 2048
M_CORE = 2048
KT = K // P          # 16
NT = N // P          # 16
MT = M_CORE // P     # 16
NBLK = N // 512      # 4
N_CORES = 8
SHARD_NT = 2         # w_shard rows / 128
C_MAGIC = 12582912.0     # 1.5 * 2**23
INV_NK = 1.0 / (N * K)
USE_COLLECTIVE = True
N_XPRO = 3           # x tiles processed on GP/ACT before the mm loop

F32 = mybir.dt.float32
BF16 = mybir.dt.bfloat16
ALU = mybir.AluOpType
AXIS = mybir.AxisListType
ACTF = mybir.ActivationFunctionType


def _build_program():
    nc = bacc.Bacc(
        "TRN2", target_bir_lowering=False, debug=False,
        num_devices=N_CORES if USE_COLLECTIVE else None,
    )

    x_d = nc.dram_tensor("x", [M_CORE, K], F32, kind="ExternalInput").ap()
    w_d = nc.dram_tensor("weight", [N, K], F32, kind="ExternalInput").ap()
    ws_d = nc.dram_tensor("w_shard", [SHARD_NT * P, K], F32, kind="ExternalInput").ap()
    b_d = nc.dram_tensor("bias", [1, N], F32, kind="ExternalInput").ap()
    out_d = nc.dram_tensor("out", [M_CORE, N], F32, kind="ExternalOutput").ap()
    ident_d = nc.inline_tensor(
        np.eye(P, dtype=ml_dtypes.bfloat16), name="ident128"
    ).ap()
    identf_d = nc.inline_tensor(
        np.eye(P, dtype=np.float32), name="ident128f"
    ).ap()

    with tile.TileContext(nc) as tc, ExitStack() as ctx:
        consts = ctx.enter_context(tc.tile_pool(name="consts", bufs=1))
        stats = ctx.enter_context(tc.tile_pool(name="stats", bufs=1))
        bias_p = ctx.enter_context(tc.tile_pool(name="bias_p", bufs=1))
        wqT_p = ctx.enter_context(tc.tile_pool(name="wqT", bufs=1))
        xqT_p = ctx.enter_context(tc.tile_pool(name="xqT", bufs=6))
        wstage = ctx.enter_context(tc.tile_pool(name="wstage", bufs=7))
        xstage = ctx.enter_context(tc.tile_pool(name="xstage", bufs=3))
        xqst = ctx.enter_context(tc.tile_pool(name="xqst", bufs=2))
        outst = ctx.enter_context(tc.tile_pool(name="outst", bufs=3))
        ps_tr = ctx.enter_context(
            tc.tile_pool(name="pstr", bufs=2, space=bass.MemorySpace.PSUM)
        )
        ps_mm = ctx.enter_context(
            tc.tile_pool(name="psmm", bufs=6, space=bass.MemorySpace.PSUM)
        )

        # ---- constants ----
        ident_sb = consts.tile([P, P], BF16)
        nc.sync.dma_start(ident_sb[:], ident_d[:, :])
        identf_sb = consts.tile([P, P], F32)
        nc.sync.dma_start(identf_sb[:], identf_d[:, :])
        ones128 = consts.tile([P, P], F32)
        nc.vector.memset(ones128[:], 1.0)
        onesrow = consts.tile([1, P], F32)
        nc.vector.memset(onesrow[:], 1.0)
        b_row = consts.tile([1, N], F32)
        nc.sync.dma_start(b_row[:], b_d[:, :])

        wparts = stats.tile([P, NT], F32)
        wsum = stats.tile([P, 1], F32)
        gamma = stats.tile([P, 1], F32)
        inv_g = stats.tile([P, 1], F32)
        neg_g = stats.tile([P, 1], F32)
        eta_raw = stats.tile([P, MT], F32)
        eta_all = stats.tile([P, MT], F32)
        inv_eta = stats.tile([P, MT], F32)
        qs_all = stats.tile([P, MT], F32)
        osc_all = stats.tile([P, MT], F32)

        bias_bcast = bias_p.tile([P, N], F32)

        # k-major quantized weights: [128 k-part, kt*2048 + n]
        wqT_all = wqT_p.tile([P, KT * N], BF16)
        wqT_3d = wqT_all[:].rearrange("p (t n) -> p t n", t=KT)

        # ============ phase S: gamma ============
        # All bulk DMA rides the two HWDGE rings (SWDGE descriptor-gen on the
        # GP Q7 cores costs ~13us per [128,2048] tile — far too slow). Ring
        # order: shard tiles + x prologue at the head, then the w stream,
        # then (in the mm loop) x-steady loads and out stores.
        x_stage_tiles = {}

        def x_dma(mt, eng):
            t = xstage.tile([P, K], F32, tag="x", name=f"x{mt}")
            eng.dma_start(t[:], x_d[mt * P:(mt + 1) * P, :])
            x_stage_tiles[mt] = t

        if USE_COLLECTIVE:
            dram = ctx.enter_context(
                tc.tile_pool(name="ccdram", bufs=1, space="DRAM")
            )
            cc_in = dram.tile([P, 1], F32)
            cc_out = dram.tile([P, 1], F32)
            for st in range(SHARD_NT):
                t = wstage.tile([P, K], F32, tag="w", name=f"wsh{st}")
                eng = nc.sync if st % 2 == 0 else nc.scalar
                eng.dma_start(t[:], ws_d[st * P:(st + 1) * P, :])
                nc.vector.tensor_reduce(
                    wparts[:, st:st + 1], t[:], axis=AXIS.X, op=ALU.add,
                    apply_absolute_value=True,
                )
            x_dma(0, nc.sync)
            x_dma(1, nc.scalar)
            x_dma(2, nc.scalar)
            nc.vector.tensor_reduce(
                wsum[:], wparts[:, 0:SHARD_NT], axis=AXIS.X, op=ALU.add
            )
            # the only SWDGE traffic: two 512B bounce transfers. The cc_out
            # readback waits on the collective (~20us) but nothing else
            # shares the GP queue, so nothing is held up.
            nc.gpsimd.dma_start(cc_in[:], wsum[:])
            nc.gpsimd.collective_compute(
                "AllReduce",
                ALU.add,
                replica_groups=[list(range(N_CORES))],
                ins=[cc_in.opt()],
                outs=[cc_out.opt()],
            )
            nc.gpsimd.dma_start(wsum[:], cc_out[:])
        else:
            for nt in range(NT):
                t = wstage.tile([P, K], F32, tag="w", name=f"wp1_{nt}")
                eng = nc.sync if nt % 2 == 0 else nc.scalar
                eng.dma_start(t[:], w_d[nt * P:(nt + 1) * P, :])
                nc.vector.tensor_reduce(
                    wparts[:, nt:nt + 1], t[:], axis=AXIS.X, op=ALU.add,
                    apply_absolute_value=True,
                )
            nc.vector.tensor_reduce(wsum[:], wparts[:], axis=AXIS.X, op=ALU.add)

        # ---- x prologue tiles (GP reduce + ACT quantize; not gamma-gated) ----
        xq_tiles = {}
        xqT_tiles = {}

        def x_head(mt, early):
            if mt in x_stage_tiles:
                t = x_stage_tiles.pop(mt)
            else:
                t = xstage.tile([P, K], F32, tag="x", name=f"x{mt}")
                eng = nc.sync if mt % 2 == 0 else nc.scalar
                eng.dma_start(t[:], x_d[mt * P:(mt + 1) * P, :])
            nc.vector.tensor_reduce(
                eta_raw[:, mt:mt + 1], t[:], axis=AXIS.X, op=ALU.max,
                apply_absolute_value=True,
            )
            nc.vector.tensor_scalar(
                eta_all[:, mt:mt + 1], eta_raw[:, mt:mt + 1],
                scalar1=1e-5, scalar2=None, op0=ALU.max,
            )
            nc.vector.reciprocal(inv_eta[:, mt:mt + 1], eta_all[:, mt:mt + 1])
            nc.vector.tensor_scalar(
                qs_all[:, mt:mt + 1], inv_eta[:, mt:mt + 1],
                scalar1=127.0, scalar2=None, op0=ALU.mult,
            )
            nc.scalar.activation(
                t[:], t[:], ACTF.Copy, bias=C_MAGIC,
                scale=qs_all[:, mt:mt + 1],
            )
            q = xqst.tile([P, K], BF16, tag="xq", name=f"xq{mt}")
            if early:
                nc.scalar.activation(q[:], t[:], ACTF.Copy, bias=-C_MAGIC)
            else:
                nc.vector.tensor_scalar(
                    q[:], t[:], scalar1=C_MAGIC, scalar2=None, op0=ALU.subtract,
                )
            xq_tiles[mt] = q

        def x_transpose(mt, early):
            q = xq_tiles.pop(mt)
            xqTm = xqT_p.tile([P, KT * P], BF16, tag="xqT", name=f"xqT{mt}")
            xqT3 = xqTm[:].rearrange("p (t m) -> p t m", t=KT)
            for g in range(4):
                pt = ps_tr.tile([P, 512], BF16, tag="ptr", name=f"xt{mt}_{g}")
                for j in range(4):
                    kt = g * 4 + j
                    nc.tensor.transpose(
                        pt[:, j * P:(j + 1) * P],
                        q[:, kt * P:(kt + 1) * P],
                        ident_sb[:],
                    )
                dst = xqT3[:, g * 4:(g + 1) * 4, :]
                src = pt[:].rearrange("p (j m) -> p j m", j=4)
                if early or g % 2 == 0:
                    nc.scalar.copy(dst, src)
                else:
                    nc.vector.tensor_copy(dst, src)
            xqT_tiles[mt] = xqT3

        for mt in range(N_XPRO):
            x_head(mt, early=True)
            x_transpose(mt, early=True)

        # ---- gamma epilogue (PE fold + scalars) ----
        pg = ps_mm.tile([P, 1], F32, tag="psmm", name="psg")
        nc.tensor.matmul(pg[:], ones128[:, :], wsum[:])
        nc.vector.tensor_scalar(
            gamma[:], pg[:], scalar1=INV_NK, scalar2=1e-5,
            op0=ALU.mult, op1=ALU.max,
        )
        nc.vector.reciprocal(inv_g[:], gamma[:])
        nc.vector.tensor_scalar(
            neg_g[:], gamma[:], scalar1=-1.0, scalar2=None, op0=ALU.mult,
        )

        # ---- bias broadcast to all partitions: psum = ones^T (x) bias ----
        for nb in range(NBLK):
            ps = ps_mm.tile([P, 512], F32, tag="psmm", name=f"psb{nb}")
            nc.tensor.matmul(
                ps[:], onesrow[:, :], b_row[:, nb * 512:(nb + 1) * 512]
            )
            nc.scalar.copy(bias_bcast[:, nb * 512:(nb + 1) * 512], ps[:])

        # ============ phase W: stream + quantize + transpose w ============
        for nt in range(NT):
            t = wstage.tile([P, K], F32, tag="w", name=f"w{nt}")
            eng = nc.sync if nt % 2 == 0 else nc.scalar
            eng.dma_start(t[:], w_d[nt * P:(nt + 1) * P, :])
            # clip(w, -g, g): round(clip(w/g,-1,1)) == round(clip(w,-g,g)/g)
            nc.vector.tensor_scalar(
                t[:], t[:], scalar1=gamma[:, :], scalar2=neg_g[:, :],
                op0=ALU.min, op1=ALU.max,
            )
            # t = w_clip * inv_g + C: fp32 store rounds to the integer grid
            nc.scalar.activation(
                t[:], t[:], ACTF.Copy, bias=C_MAGIC, scale=inv_g[:, :]
            )
            for g in range(4):
                pt = ps_tr.tile([P, 512], F32, tag="ptr", name=f"wt{nt}_{g}")
                for j in range(4):
                    kt = g * 4 + j
                    nc.tensor.transpose(
                        pt[:, j * P:(j + 1) * P],
                        t[:, kt * P:(kt + 1) * P],
                        identf_sb[:],
                    )
                dst = wqT_3d[:, g * 4:(g + 1) * 4, nt * P:(nt + 1) * P]
                src = pt[:].rearrange("p (j n) -> p j n", j=4)
                if g % 3 == 0:
                    nc.scalar.activation(dst, src, ACTF.Copy, bias=-C_MAGIC)
                else:
                    nc.vector.tensor_scalar(
                        dst, src, scalar1=C_MAGIC, scalar2=None,
                        op0=ALU.subtract,
                    )

        # ============ phase MM: single mt-outer sweep ============
        def mm_block(mt):
            nc.vector.tensor_scalar(
                osc_all[:, mt:mt + 1], eta_all[:, mt:mt + 1],
                scalar1=gamma[:, :], scalar2=1.0 / 127.0,
                op0=ALU.mult, op1=ALU.mult,
            )
            xqT3 = xqT_tiles.pop(mt)
            pss = [
                ps_mm.tile([P, 512], F32, tag="psmm", name=f"ps{mt}_{nb}")
                for nb in range(NBLK)
            ]
            for kt in range(KT):
                lhsT = xqT3[:, kt, :]
                for nb in range(NBLK):
                    nc.tensor.matmul(
                        pss[nb][:],
                        lhsT,
                        wqT_3d[:, kt, nb * 512:(nb + 1) * 512],
                        start=(kt == 0),
                        stop=(kt == KT - 1),
                    )
            for nb in range(NBLK):
                o = outst.tile([P, 512], F32, tag="o", name=f"o{mt}_{nb}")
                nc.vector.scalar_tensor_tensor(
                    o[:], pss[nb][:], osc_all[:, mt:mt + 1],
                    bias_bcast[:, nb * 512:(nb + 1) * 512],
                    op0=ALU.mult, op1=ALU.add,
                )
                eng = nc.sync if nb % 2 == 0 else nc.scalar
                eng.dma_start(
                    out_d[mt * P:(mt + 1) * P, nb * 512:(nb + 1) * 512], o[:]
                )

        for mt in range(MT):
            nxt = mt + N_XPRO
            if nxt < MT:
                x_head(nxt, early=False)
                x_transpose(nxt, early=False)
            mm_block(mt)
    nc.compile()
    return nc


_NC_CACHE = None
LAST_EXEC_NS = None


def _get_nc():
    global _NC_CACHE
    if _NC_CACHE is None:
        _NC_CACHE = _build_program()
    return _NC_CACHE


def _make_in_maps(x, weight, bias):
    xf = np.ascontiguousarray(np.asarray(x, dtype=np.float32).reshape(-1, K))
    w = np.ascontiguousarray(np.asarray(weight, dtype=np.float32))
    b = np.ascontiguousarray(np.asarray(bias, dtype=np.float32).reshape(1, N))
    assert xf.shape[0] == N_CORES * M_CORE
    rows = SHARD_NT * P
    return [
        {
            "x": xf[c * M_CORE:(c + 1) * M_CORE],
            "weight": w,
            "w_shard": np.ascontiguousarray(w[c * rows:(c + 1) * rows]),
            "bias": b,
        }
        for c in range(N_CORES)
    ]


def kernel(x, weight, bias):
    global LAST_EXEC_NS
    nc = _get_nc()
    in_maps = _make_in_maps(x, weight, bias)
    trace = bool(int(os.environ.get("BITLINEAR_TRACE", "0")))
    res = run_bass_kernel_spmd(nc, in_maps, list(range(N_CORES)), trace=trace)
    LAST_EXEC_NS = res.exec_time_ns
    out = np.concatenate([res.results[c]["out"] for c in range(N_CORES)], axis=0)
    return out.reshape(np.asarray(x).shape[:-1] + (N,)).astype(np.float32)


# revision 14
# speedup vs baseline: 1.3026x; 1.0235x over previous
"""BitLinear (ternary-weight / int8-activation quantized linear) on 8 trn2 NeuronCores.

Math (matches the jax reference up to fp32 rounding):
    eta   = clip(max|x| along k, 1e-5)             per row
    x_q   = round(x * 127 / eta)    in [-127,127]  (round-half-even)
    gamma = clip(mean|w|, 1e-5)                    scalar
    w_q   = round(clip(w / gamma, -1, 1))          in {-1,0,1}
    out   = (x_q @ w_q^T) * (eta/127 * gamma) + bias

x_q / w_q are small integers exactly representable in bf16 and the PE
accumulates in fp32, so the bf16 matmul is EXACT.  Rounding uses the fp32
magic-number trick  rint(t) = (t + 1.5*2^23) - 1.5*2^23  (round-half-even).
The w clip is done BEFORE scaling:  round(clip(w/g,-1,1)) == round(clip(w,-g,g)/g)
for any g>0 (elements |w|>=g map to +-1 either way), which saves a full pass.

Sharding: data-parallel over rows of x (16384 -> 2048 rows/core), weight+bias
replicated.  gamma = mean|w| is computed with an 8-way AllReduce collective:
each core |.|-reduces only its 256-row shard of w (host-sliced input w_shard),
so the full-w stream only has to happen ONCE per core and the scalar gamma is
ready ~25us in instead of ~50us.

Per-core schedule (all phases overlap):
  S:  DMA w_shard (2 tiles), |.|-reduce, AllReduce -> gamma, inv_g, -gamma
  W:  stream all 16 w tiles once (HWDGE, starts at t=0, buffered bufs=7);
      per tile: clip to [-g,g] (DVE), *inv_g + C magic-round (ACT),
      PE fp32-transpose, PSUM->SBUF copy with fused -C -> wqT bf16
  X:  per tile: DMA (SWDGE), abs-max -> eta (GP early / DVE late), *qs + C
      (ACT), -C -> bf16 (ACT/DVE), PE bf16-transpose -> rolling xqT pool
  MM: single mt-outer sweep, 16 kt x 4 nb bf16 matmuls -> 4 PSUM banks,
      fused dequant+bias (scalar_tensor_tensor: psum*osc + bias_bcast) -> out
"""

import os
from contextlib import ExitStack

import numpy as np
import ml_dtypes

import concourse.bass as bass
import concourse.bacc as bacc
import concourse.mybir as mybir
import concourse.tile as tile
from concourse.bass_utils import run_bass_kernel_spmd

P = 128
K = 2048
N = 2048
M_CORE = 2048
KT = K // P          # 16
NT = N // P          # 16
MT = M_CORE // P     # 16
NBLK = N // 512      # 4
N_CORES = 8
SHARD_NT = 2         # w_shard rows / 128
C_MAGIC = 12582912.0     # 1.5 * 2**23
INV_NK = 1.0 / (N * K)
USE_COLLECTIVE = False   # 8-core AllReduce gamma: the collective itself takes
                         # ~4us but is gated behind an all-core entry sync
                         # (~70us on HW) -> net loss vs streaming w twice.
N_XPRO = 3           # x tiles processed on GP/ACT before the mm loop
WSTAGE_BUFS = 7      # w tiles buffered; last 7 of pass 1 stay resident

F32 = mybir.dt.float32
BF16 = mybir.dt.bfloat16
ALU = mybir.AluOpType
AXIS = mybir.AxisListType
ACTF = mybir.ActivationFunctionType


def _build_program():
    nc = bacc.Bacc(
        "TRN2", target_bir_lowering=False, debug=False,
        num_devices=N_CORES if USE_COLLECTIVE else None,
    )

    x_d = nc.dram_tensor("x", [M_CORE, K], F32, kind="ExternalInput").ap()
    w_d = nc.dram_tensor("weight", [N, K], F32, kind="ExternalInput").ap()
    ws_d = nc.dram_tensor("w_shard", [SHARD_NT * P, K], F32, kind="ExternalInput").ap()
    b_d = nc.dram_tensor("bias", [1, N], F32, kind="ExternalInput").ap()
    out_d = nc.dram_tensor("out", [M_CORE, N], F32, kind="ExternalOutput").ap()
    ident_d = nc.inline_tensor(
        np.eye(P, dtype=ml_dtypes.bfloat16), name="ident128"
    ).ap()
    identf_d = nc.inline_tensor(
        np.eye(P, dtype=np.float32), name="ident128f"
    ).ap()

    with tile.TileContext(nc) as tc, ExitStack() as ctx:
        consts = ctx.enter_context(tc.tile_pool(name="consts", bufs=1))
        stats = ctx.enter_context(tc.tile_pool(name="stats", bufs=1))
        bias_p = ctx.enter_context(tc.tile_pool(name="bias_p", bufs=1))
        wqT_p = ctx.enter_context(tc.tile_pool(name="wqT", bufs=1))
        xqT_p = ctx.enter_context(tc.tile_pool(name="xqT", bufs=6))
        wstage = ctx.enter_context(tc.tile_pool(name="wstage", bufs=7))
        xstage = ctx.enter_context(tc.tile_pool(name="xstage", bufs=3))
        xqst = ctx.enter_context(tc.tile_pool(name="xqst", bufs=2))
        outst = ctx.enter_context(tc.tile_pool(name="outst", bufs=3))
        ps_tr = ctx.enter_context(
            tc.tile_pool(name="pstr", bufs=2, space=bass.MemorySpace.PSUM)
        )
        ps_mm = ctx.enter_context(
            tc.tile_pool(name="psmm", bufs=6, space=bass.MemorySpace.PSUM)
        )

        # ---- constants ----
        ident_sb = consts.tile([P, P], BF16)
        nc.sync.dma_start(ident_sb[:], ident_d[:, :])
        identf_sb = consts.tile([P, P], F32)
        nc.sync.dma_start(identf_sb[:], identf_d[:, :])
        ones128 = consts.tile([P, P], F32)
        nc.vector.memset(ones128[:], 1.0)
        onesrow = consts.tile([1, P], F32)
        nc.vector.memset(onesrow[:], 1.0)
        b_row = consts.tile([1, N], F32)
        nc.sync.dma_start(b_row[:], b_d[:, :])

        wparts = stats.tile([P, NT], F32)
        wsum = stats.tile([P, 1], F32)
        gamma = stats.tile([P, 1], F32)
        inv_g = stats.tile([P, 1], F32)
        neg_g = stats.tile([P, 1], F32)
        eta_raw = stats.tile([P, MT], F32)
        eta_all = stats.tile([P, MT], F32)
        inv_eta = stats.tile([P, MT], F32)
        qs_all = stats.tile([P, MT], F32)
        osc_all = stats.tile([P, MT], F32)

        bias_bcast = bias_p.tile([P, N], F32)

        # k-major quantized weights: [128 k-part, kt*2048 + n]
        wqT_all = wqT_p.tile([P, KT * N], BF16)
        wqT_3d = wqT_all[:].rearrange("p (t n) -> p t n", t=KT)

        # ============ phase S: gamma ============
        # All bulk DMA rides the two HWDGE rings (SWDGE descriptor-gen on the
        # GP Q7 cores costs ~13us per [128,2048] tile — far too slow). Ring
        # order: shard tiles + x prologue at the head, then the w stream,
        # then (in the mm loop) x-steady loads and out stores.
        x_stage_tiles = {}

        def x_dma(mt, eng):
            t = xstage.tile([P, K], F32, tag="x", name=f"x{mt}")
            eng.dma_start(t[:], x_d[mt * P:(mt + 1) * P, :])
            x_stage_tiles[mt] = t

        w_resident = {}
        if USE_COLLECTIVE:
            dram = ctx.enter_context(
                tc.tile_pool(name="ccdram", bufs=1, space="DRAM")
            )
            cc_in = dram.tile([P, 1], F32)
            cc_out = dram.tile([P, 1], F32)
            for st in range(SHARD_NT):
                t = wstage.tile([P, K], F32, tag="w", name=f"wsh{st}")
                eng = nc.sync if st % 2 == 0 else nc.scalar
                eng.dma_start(t[:], ws_d[st * P:(st + 1) * P, :])
                nc.vector.tensor_reduce(
                    wparts[:, st:st + 1], t[:], axis=AXIS.X, op=ALU.add,
                    apply_absolute_value=True,
                )
            x_dma(0, nc.sync)
            x_dma(1, nc.scalar)
            x_dma(2, nc.scalar)
            nc.vector.tensor_reduce(
                wsum[:], wparts[:, 0:SHARD_NT], axis=AXIS.X, op=ALU.add
            )
            # the only SWDGE traffic: two 512B bounce transfers. The cc_out
            # readback waits on the collective (~20us) but nothing else
            # shares the GP queue, so nothing is held up.
            nc.gpsimd.dma_start(cc_in[:], wsum[:])
            nc.gpsimd.collective_compute(
                "AllReduce",
                ALU.add,
                replica_groups=[list(range(N_CORES))],
                ins=[cc_in.opt()],
                outs=[cc_out.opt()],
            )
            nc.gpsimd.dma_start(wsum[:], cc_out[:])
        else:
            x_dma(0, nc.sync)
            x_dma(1, nc.scalar)
            x_dma(2, nc.scalar)
            # pass 1: stream all of w for gamma. The last WSTAGE_BUFS tiles
            # stay buffered (their pool slots aren't recycled until the
            # restream reaches them), so phase W can reuse them without DMA.
            for nt in range(NT):
                t = wstage.tile([P, K], F32, tag="w", name=f"wp1_{nt}")
                eng = nc.sync if nt % 2 == 0 else nc.scalar
                eng.dma_start(t[:], w_d[nt * P:(nt + 1) * P, :])
                nc.vector.tensor_reduce(
                    wparts[:, nt:nt + 1], t[:], axis=AXIS.X, op=ALU.add,
                    apply_absolute_value=True,
                )
                if nt >= NT - WSTAGE_BUFS:
                    w_resident[nt] = t
            nc.vector.tensor_reduce(wsum[:], wparts[:], axis=AXIS.X, op=ALU.add)

        # ---- x prologue tiles (GP reduce + ACT quantize; not gamma-gated) ----
        xq_tiles = {}
        xqT_tiles = {}

        def x_head(mt, early):
            if mt in x_stage_tiles:
                t = x_stage_tiles.pop(mt)
            else:
                t = xstage.tile([P, K], F32, tag="x", name=f"x{mt}")
                eng = nc.sync if mt % 2 == 0 else nc.scalar
                eng.dma_start(t[:], x_d[mt * P:(mt + 1) * P, :])
            nc.vector.tensor_reduce(
                eta_raw[:, mt:mt + 1], t[:], axis=AXIS.X, op=ALU.max,
                apply_absolute_value=True,
            )
            nc.vector.tensor_scalar(
                eta_all[:, mt:mt + 1], eta_raw[:, mt:mt + 1],
                scalar1=1e-5, scalar2=None, op0=ALU.max,
            )
            nc.vector.reciprocal(inv_eta[:, mt:mt + 1], eta_all[:, mt:mt + 1])
            nc.vector.tensor_scalar(
                qs_all[:, mt:mt + 1], inv_eta[:, mt:mt + 1],
                scalar1=127.0, scalar2=None, op0=ALU.mult,
            )
            nc.scalar.activation(
                t[:], t[:], ACTF.Copy, bias=C_MAGIC,
                scale=qs_all[:, mt:mt + 1],
            )
            q = xqst.tile([P, K], BF16, tag="xq", name=f"xq{mt}")
            if early:
                nc.scalar.activation(q[:], t[:], ACTF.Copy, bias=-C_MAGIC)
            else:
                nc.vector.tensor_scalar(
                    q[:], t[:], scalar1=C_MAGIC, scalar2=None, op0=ALU.subtract,
                )
            xq_tiles[mt] = q

        def x_transpose(mt, early):
            q = xq_tiles.pop(mt)
            xqTm = xqT_p.tile([P, KT * P], BF16, tag="xqT", name=f"xqT{mt}")
            xqT3 = xqTm[:].rearrange("p (t m) -> p t m", t=KT)
            for g in range(4):
                pt = ps_tr.tile([P, 512], BF16, tag="ptr", name=f"xt{mt}_{g}")
                for j in range(4):
                    kt = g * 4 + j
                    nc.tensor.transpose(
                        pt[:, j * P:(j + 1) * P],
                        q[:, kt * P:(kt + 1) * P],
                        ident_sb[:],
                    )
                dst = xqT3[:, g * 4:(g + 1) * 4, :]
                src = pt[:].rearrange("p (j m) -> p j m", j=4)
                if early or g % 2 == 0:
                    nc.scalar.copy(dst, src)
                else:
                    nc.vector.tensor_copy(dst, src)
            xqT_tiles[mt] = xqT3

        for mt in range(N_XPRO):
            x_head(mt, early=True)
            x_transpose(mt, early=True)

        # ---- gamma epilogue (PE fold + scalars) ----
        pg = ps_mm.tile([P, 1], F32, tag="psmm", name="psg")
        nc.tensor.matmul(pg[:], ones128[:, :], wsum[:])
        nc.vector.tensor_scalar(
            gamma[:], pg[:], scalar1=INV_NK, scalar2=1e-5,
            op0=ALU.mult, op1=ALU.max,
        )
        nc.vector.reciprocal(inv_g[:], gamma[:])
        nc.vector.tensor_scalar(
            neg_g[:], gamma[:], scalar1=-1.0, scalar2=None, op0=ALU.mult,
        )

        # ---- bias broadcast to all partitions: psum = ones^T (x) bias ----
        for nb in range(NBLK):
            ps = ps_mm.tile([P, 512], F32, tag="psmm", name=f"psb{nb}")
            nc.tensor.matmul(
                ps[:], onesrow[:, :], b_row[:, nb * 512:(nb + 1) * 512]
            )
            nc.scalar.copy(bias_bcast[:, nb * 512:(nb + 1) * 512], ps[:])

        # ============ phase W: stream + quantize + transpose w ============
        # resident tiles (already in SBUF from pass 1) first, then the
        # restreamed ones in arrival order.
        w_order = sorted(w_resident) + [nt for nt in range(NT) if nt not in w_resident]
        for wi, nt in enumerate(w_order):
            if nt in w_resident:
                t = w_resident.pop(nt)
            else:
                t = wstage.tile([P, K], F32, tag="w", name=f"w{nt}")
                eng = nc.sync if nt % 2 == 0 else nc.scalar
                eng.dma_start(t[:], w_d[nt * P:(nt + 1) * P, :])
            # clip(w, -g, g): round(clip(w/g,-1,1)) == round(clip(w,-g,g)/g)
            clip_eng = nc.vector if wi % 2 == 0 else nc.gpsimd
            clip_eng.tensor_scalar(
                t[:], t[:], scalar1=gamma[:, :], scalar2=neg_g[:, :],
                op0=ALU.min, op1=ALU.max,
            )
            # t = w_clip * inv_g + C: fp32 store rounds to the integer grid
            nc.scalar.activation(
                t[:], t[:], ACTF.Copy, bias=C_MAGIC, scale=inv_g[:, :]
            )
            for g in range(4):
                pt = ps_tr.tile([P, 512], F32, tag="ptr", name=f"wt{nt}_{g}")
                for j in range(4):
                    kt = g * 4 + j
                    nc.tensor.transpose(
                        pt[:, j * P:(j + 1) * P],
                        t[:, kt * P:(kt + 1) * P],
                        identf_sb[:],
                    )
                dst = wqT_3d[:, g * 4:(g + 1) * 4, nt * P:(nt + 1) * P]
                src = pt[:].rearrange("p (j n) -> p j n", j=4)
                if g % 3 == 0:
                    nc.scalar.activation(dst, src, ACTF.Copy, bias=-C_MAGIC)
                else:
                    nc.vector.tensor_scalar(
                        dst, src, scalar1=C_MAGIC, scalar2=None,
                        op0=ALU.subtract,
                    )

        # ============ phase MM: single mt-outer sweep ============
        def mm_block(mt):
            nc.vector.tensor_scalar(
                osc_all[:, mt:mt + 1], eta_all[:, mt:mt + 1],
                scalar1=gamma[:, :], scalar2=1.0 / 127.0,
                op0=ALU.mult, op1=ALU.mult,
            )
            xqT3 = xqT_tiles.pop(mt)
            pss = [
                ps_mm.tile([P, 512], F32, tag="psmm", name=f"ps{mt}_{nb}")
                for nb in range(NBLK)
            ]
            for kt in range(KT):
                lhsT = xqT3[:, kt, :]
                for nb in range(NBLK):
                    nc.tensor.matmul(
                        pss[nb][:],
                        lhsT,
                        wqT_3d[:, kt, nb * 512:(nb + 1) * 512],
                        start=(kt == 0),
                        stop=(kt == KT - 1),
                    )
            for nb in range(NBLK):
                o = outst.tile([P, 512], F32, tag="o", name=f"o{mt}_{nb}")
                nc.vector.scalar_tensor_tensor(
                    o[:], pss[nb][:], osc_all[:, mt:mt + 1],
                    bias_bcast[:, nb * 512:(nb + 1) * 512],
                    op0=ALU.mult, op1=ALU.add,
                )
                eng = nc.sync if nb % 2 == 0 else nc.scalar
                eng.dma_start(
                    out_d[mt * P:(mt + 1) * P, nb * 512:(nb + 1) * 512], o[:]
                )

        for mt in range(MT):
            nxt = mt + N_XPRO
            if nxt < MT:
                x_head(nxt, early=False)
                x_transpose(nxt, early=False)
            mm_block(mt)
    nc.compile()
    return nc


_NC_CACHE = None
LAST_EXEC_NS = None


def _get_nc():
    global _NC_CACHE
    if _NC_CACHE is None:
        _NC_CACHE = _build_program()
    return _NC_CACHE


def _make_in_maps(x, weight, bias):
    xf = np.ascontiguousarray(np.asarray(x, dtype=np.float32).reshape(-1, K))
    w = np.ascontiguousarray(np.asarray(weight, dtype=np.float32))
    b = np.ascontiguousarray(np.asarray(bias, dtype=np.float32).reshape(1, N))
    assert xf.shape[0] == N_CORES * M_CORE
    rows = SHARD_NT * P
    return [
        {
            "x": xf[c * M_CORE:(c + 1) * M_CORE],
            "weight": w,
            "w_shard": np.ascontiguousarray(w[c * rows:(c + 1) * rows]),
            "bias": b,
        }
        for c in range(N_CORES)
    ]


def kernel(x, weight, bias):
    global LAST_EXEC_NS
    nc = _get_nc()
    in_maps = _make_in_maps(x, weight, bias)
    trace = bool(int(os.environ.get("BITLINEAR_TRACE", "0")))
    res = run_bass_kernel_spmd(nc, in_maps, list(range(N_CORES)), trace=trace)
    LAST_EXEC_NS = res.exec_time_ns
    out = np.concatenate([res.results[c]["out"] for c in range(N_CORES)], axis=0)
    return out.reshape(np.asarray(x).shape[:-1] + (N,)).astype(np.float32)
